# revision 15
# baseline (speedup 1.0000x reference)
"""ContrastiveLoss Trainium2 kernel (8 NeuronCores, SPMD).

Math: reference computes, over p = predict.reshape(B, D):
    d2_ij  = max(||p_i||^2 + ||p_j||^2 - 2 p_i.p_j, 0) / D
    lab_ij = [g_i == g_j]
    term   = lab*d2 + (1-lab)*relu(M - d2),  M = 2
    loss   = 2 * sum_{j>=i} term / (B*(B-1))

Device formulation (full matrix, diagonal self-cancelling):
  q = bf16(sqrt(2)*p); stil_i = (q_i.q_i)/2  (host fp32)
  PSUM_A = q_i.q_j - (stil_j - C)            (bf16 GEMM K=2048 + K=2 feature GEMM)
  d2     = relu(-PSUM_A/D + (stil_i + C)/D)  == ||p^_i - p^_j||^2/D  >= 0 exactly
  t      = relu(M - d2)                      (scalar engine, accum_out -> col sums)
  PSUM_B = dg2 - 2/LAM  where dg2 = sum_digit (d_i - d_j)^2  (exact bf16 GEMM K=10)
  w      = relu((d2 - t) + 2 - LAM*dg2)      (vector engine, accum_out -> col sums)
  sum over full matrix:  S = sum t + sum w  satisfies
  loss   = (S - 2*N_label) / (B*(B-1)),  N_label = sum_c count_c^2 (ordered, incl diag)
Each core owns 1024 rows x all 8192 cols; host sums 8x2 [128,128] accumulators.
"""

import os
import math
from contextlib import ExitStack

import numpy as np
import ml_dtypes

BF16 = ml_dtypes.bfloat16

B = 8192
DFEAT = 2048
NCORES = 8
P = 128
WINW = 512
MARGIN = 2.0
C_OFF = 2048.0
LAM = 64.0

USE_FP8 = True

LAST_EXEC_NS = None
LAST_RESULTS = None


def tri_entries(ncores=NCORES, nblks=16):
    """Per-core list of (row_block, col_block) in 512-units: 2 diagonal entries
    first, then 15 plain upper-triangle entries. Uniform length for every core."""
    per_core = []
    for k in range(ncores):
        a, bb = k, nblks - 1 - k
        ents = [(a, a), (bb, bb)]
        ents += [(a, c) for c in range(a + 1, nblks)]
        ents += [(bb, c) for c in range(bb + 1, nblks)]
        assert len(ents) == nblks + 1
        per_core.append(ents)
    return per_core


def build_tri_nc(b=B, dfeat=DFEAT, ncores=NCORES, use_fp8=False):
    """Triangle variant: 17 host-packed (lhs, rhs) stream entries per core,
    4 row-tiles each; entries 0,1 are diagonal blocks masked to j > i.
    use_fp8: main GEMM in float8e4 with DoubleRow (2 K-chunks per matmul)."""
    import concourse.bacc as bacc
    import concourse.mybir as mybir
    from concourse.tile import TileContext

    fp32 = mybir.dt.float32
    bf16 = mybir.dt.bfloat16
    qdt = mybir.dt.float8e4 if use_fp8 else bf16
    Act = mybir.ActivationFunctionType
    Alu = mybir.AluOpType

    kch = dfeat // P               # contraction chunks
    nent = (b // WINW) + 1         # 17 stream entries
    rt_n = WINW // P               # 4 row tiles per entry
    nblk = nent * rt_n             # 68 accumulator columns

    nc = bacc.Bacc()
    lhs_s = nc.declare_dram_parameter("lhs_s", [nent, dfeat, WINW], qdt, isOutput=False)
    rhs_s = nc.declare_dram_parameter("rhs_s", [nent, dfeat, WINW], qdt, isOutput=False)
    sqf_s = nc.declare_dram_parameter("sqf_s", [nent, 2, WINW], bf16, isOutput=False)
    labr_s = nc.declare_dram_parameter("labr_s", [nent, 10, WINW], bf16, isOutput=False)
    labl_s = nc.declare_dram_parameter("labl_s", [nent, 10, WINW], bf16, isOutput=False)
    bias_s = nc.declare_dram_parameter("bias_s", [nent, P, rt_n], fp32, isOutput=False)
    acct_d = nc.declare_dram_parameter("acc_t", [P, nblk], fp32, isOutput=True)
    accw_d = nc.declare_dram_parameter("acc_w", [P, nblk], fp32, isOutput=True)

    with TileContext(nc) as tc, ExitStack() as ctx:
        const = ctx.enter_context(tc.tile_pool(name="const", bufs=1))
        str_pool = ctx.enter_context(tc.tile_pool(name="streams", bufs=3))
        sc_pool = ctx.enter_context(tc.tile_pool(name="scratch", bufs=3))
        psA_pool = ctx.enter_context(tc.tile_pool(name="psA", bufs=2, space="PSUM"))
        psB_pool = ctx.enter_context(tc.tile_pool(name="psB", bufs=2, space="PSUM"))
        acc_pool = ctx.enter_context(tc.tile_pool(name="acc", bufs=1))

        ones2 = const.tile([2, P], bf16)
        nc.any.memset(ones2[:], 1.0)
        mbias = const.tile([P, 1], fp32)
        nc.any.memset(mbias[:], MARGIN)
        acct_t = acc_pool.tile([P, nblk], fp32)
        accw_t = acc_pool.tile([P, nblk], fp32)

        lhs_r = lhs_s.rearrange("e (c p) n -> e p c n", p=P)
        rhs_r = rhs_s.rearrange("e (c p) n -> e p c n", p=P)

        for e in range(nent):
            lhs_t = str_pool.tile([P, kch, WINW], qdt, tag="lhs")
            nc.sync.dma_start(out=lhs_t[:], in_=lhs_r[e])
            rhs_t = str_pool.tile([P, kch, WINW], qdt, tag="rhs")
            nc.sync.dma_start(out=rhs_t[:], in_=rhs_r[e])
            sqf_t = str_pool.tile([2, WINW], bf16, tag="sqf")
            nc.sync.dma_start(out=sqf_t[:], in_=sqf_s[e])
            labr_t = str_pool.tile([10, WINW], bf16, tag="labr")
            nc.sync.dma_start(out=labr_t[:], in_=labr_s[e])
            labl_t = str_pool.tile([10, WINW], bf16, tag="labl")
            nc.sync.dma_start(out=labl_t[:], in_=labl_s[e])
            bias_t = str_pool.tile([P, rt_n], fp32, tag="bias")
            nc.sync.dma_start(out=bias_t[:], in_=bias_s[e])
            for r in range(rt_n):
                blk = e * rt_n + r
                ms = slice(r * P, (r + 1) * P)
                psA = psA_pool.tile([P, WINW], fp32, tag="psA")
                psB = psB_pool.tile([P, WINW], fp32, tag="psB")
                if use_fp8:
                    for c in range(0, kch, 2):
                        nc.tensor.matmul(
                            psA[:], lhs_t[:, c : c + 2, ms], rhs_t[:, c : c + 2, :],
                            start=(c == 0), stop=False,
                            perf_mode=mybir.MatmulPerfMode.DoubleRow,
                        )
                else:
                    for c in range(kch):
                        nc.tensor.matmul(
                            psA[:], lhs_t[:, c, ms], rhs_t[:, c, :],
                            start=(c == 0), stop=False,
                        )
                nc.tensor.matmul(psA[:], ones2[:], sqf_t[:], start=False, stop=True)
                nc.tensor.matmul(psB[:], labl_t[:, ms], labr_t[:], start=True, stop=True)

                d2 = sc_pool.tile([P, WINW], fp32, tag="d2")
                nc.scalar.activation(
                    d2[:], psA[:], Act.Relu,
                    bias=bias_t[:, r : r + 1], scale=-1.0 / dfeat,
                )
                if e < 2:
                    # diagonal block: keep strictly-upper cells (col > row),
                    # zero the rest; host corrects the t=M per zeroed cell.
                    d2m = sc_pool.tile([P, WINW], fp32, tag="d2m")
                    nc.gpsimd.affine_select(
                        d2m[:], d2[:], pattern=[[1, WINW]],
                        compare_op=Alu.is_gt, fill=0.0,
                        base=-(r * P), channel_multiplier=-1,
                    )
                    d2 = d2m
                t_ = sc_pool.tile([P, WINW], fp32, tag="t")
                nc.scalar.activation(
                    t_[:], d2[:], Act.Relu,
                    bias=mbias[:], scale=-1.0,
                    accum_out=acct_t[:, blk : blk + 1],
                )
                u_ = sc_pool.tile([P, WINW], fp32, tag="u")
                nc.vector.scalar_tensor_tensor(
                    u_[:], in0=t_[:], scalar=-1.0, in1=d2[:],
                    op0=Alu.mult, op1=Alu.add,
                )
                y_ = sc_pool.tile([P, WINW], fp32, tag="y")
                nc.vector.scalar_tensor_tensor(
                    y_[:], in0=psB[:], scalar=-LAM, in1=u_[:],
                    op0=Alu.mult, op1=Alu.add,
                )
                w_ = sc_pool.tile([P, WINW], fp32, tag="w")
                nc.vector.tensor_scalar(
                    w_[:], y_[:], 0.0, None, Alu.max, Alu.add,
                    accum_out=accw_t[:, blk : blk + 1],
                )

        nc.sync.dma_start(out=acct_d[:], in_=acct_t[:])
        nc.sync.dma_start(out=accw_d[:], in_=accw_t[:])
    nc.compile()
    return nc


def host_prep_tri(predict, gt, b=B, dfeat=DFEAT, ncores=NCORES, use_fp8=False):
    """Per-core input maps for the triangle variant + host correction counts.

    use_fp8: quantize q to float8_e4m3 and debias the squared norms by the
    known quantization error energy, so E[d2_hat] == d2 (the raw fp8
    ||p^_i - p^_j||^2 overshoots by (||e_i||^2+||e_j||^2)/D otherwise)."""
    p = np.asarray(predict, np.float32).reshape(b, dfeat)
    q = p * np.float32(math.sqrt(2.0))
    qb = q.astype(ml_dtypes.float8_e4m3 if use_fp8 else BF16)
    qf = qb.astype(np.float32)
    if use_fp8:
        # true norms: cancels the row-common part of the quantization error
        # (E[q.e] != 0 for coarse RN grids); diagonal cells are masked anyway.
        stil = (0.5 * np.einsum("ij,ij->i", q.astype(np.float64), q.astype(np.float64))).astype(np.float32)
    else:
        stil = (0.5 * np.einsum("ij,ij->i", qf.astype(np.float64), qf.astype(np.float64))).astype(np.float32)
    smc = (stil - np.float32(C_OFF)).astype(np.float32)
    s_hi = smc.astype(BF16)
    s_lo = (smc - s_hi.astype(np.float32)).astype(BF16)
    sqf_full = np.stack([-s_hi, -s_lo]).astype(BF16)

    g = np.asarray(gt).reshape(-1).astype(np.int64)
    da, db_, dc = g % 10, (g // 10) % 10, g // 100
    onesb = np.ones(b, np.float32)
    labr_full = np.stack(
        [onesb, da, da * da, onesb, db_, db_ * db_, onesb, dc, dc * dc, onesb]
    ).astype(BF16)
    labl_full = np.stack(
        [da * da, -2.0 * da, onesb, db_ * db_, -2.0 * db_, onesb,
         dc * dc, -2.0 * dc, onesb, np.full(b, -2.0 / LAM, np.float32)]
    ).astype(BF16)
    bias_full = ((stil + np.float32(C_OFF)) / np.float32(dfeat)).astype(np.float32)
    qT = np.ascontiguousarray(qb.T)

    nblks = b // WINW
    del q, qf
    rt_n = WINW // P
    entries = tri_entries(ncores, nblks)
    in_maps = []
    for k in range(ncores):
        ents = entries[k]
        lhs_sx = np.stack([qT[:, rb * WINW:(rb + 1) * WINW] for rb, _ in ents])
        rhs_sx = np.stack([qT[:, cb * WINW:(cb + 1) * WINW] for _, cb in ents])
        sqf_sx = np.stack([sqf_full[:, cb * WINW:(cb + 1) * WINW] for _, cb in ents])
        labr_sx = np.stack([labr_full[:, cb * WINW:(cb + 1) * WINW] for _, cb in ents])
        labl_sx = np.stack([labl_full[:, rb * WINW:(rb + 1) * WINW] for rb, _ in ents])
        bias_sx = np.stack([
            np.ascontiguousarray(bias_full[rb * WINW:(rb + 1) * WINW].reshape(rt_n, P).T)
            for rb, _ in ents])
        in_maps.append({
            "lhs_s": np.ascontiguousarray(lhs_sx),
            "rhs_s": np.ascontiguousarray(rhs_sx),
            "sqf_s": np.ascontiguousarray(sqf_sx),
            "labr_s": np.ascontiguousarray(labr_sx),
            "labl_s": np.ascontiguousarray(labl_sx),
            "bias_s": np.ascontiguousarray(bias_sx),
        })
    n_label = int((np.bincount(g) ** 2).sum())
    n_masked = ncores * 2 * (WINW * (WINW + 1) // 2)   # j<=i cells zeroed per diag entry
    return in_maps, n_label, n_masked


def finish_tri(results, n_label, n_masked, b=B):
    s = 0.0
    for r in results:
        s += float(r["acc_t"].astype(np.float64).sum())
        s += float(r["acc_w"].astype(np.float64).sum())
    n_lab_strict = (n_label - b) // 2
    s_strict = s - MARGIN * n_masked - 2.0 * n_lab_strict
    loss = 2.0 * s_strict / (float(b) * (b - 1))
    return np.float32(loss)


def build_nc(b=B, dfeat=DFEAT, ncores=NCORES, nwin_override=None):
    import concourse.bass as bass
    import concourse.bacc as bacc
    import concourse.mybir as mybir
    from concourse.tile import TileContext

    fp32 = mybir.dt.float32
    bf16 = mybir.dt.bfloat16
    Act = mybir.ActivationFunctionType
    Alu = mybir.AluOpType

    rpc = b // ncores              # rows per core
    rt_n = rpc // P                # row tiles per core
    nwin = b // WINW               # column windows
    nwin_run = nwin if nwin_override is None else nwin_override
    kch = dfeat // P               # contraction chunks
    nblk = nwin * rt_n

    nc = bacc.Bacc()
    qT = nc.declare_dram_parameter("qT", [dfeat, b], bf16, isOutput=False)
    lhsT = nc.declare_dram_parameter("lhsT", [dfeat, rpc], bf16, isOutput=False)
    sqf = nc.declare_dram_parameter("sqf", [2, b], bf16, isOutput=False)
    labr = nc.declare_dram_parameter("labr", [10, b], bf16, isOutput=False)
    labl = nc.declare_dram_parameter("labl", [10, rpc], bf16, isOutput=False)
    bias = nc.declare_dram_parameter("bias", [P, rt_n], fp32, isOutput=False)
    acct_d = nc.declare_dram_parameter("acc_t", [P, nblk], fp32, isOutput=True)
    accw_d = nc.declare_dram_parameter("acc_w", [P, nblk], fp32, isOutput=True)

    with TileContext(nc) as tc, ExitStack() as ctx:
        const = ctx.enter_context(tc.tile_pool(name="const", bufs=1))
        lhs_pool = ctx.enter_context(tc.tile_pool(name="lhs", bufs=1))
        rhs_pool = ctx.enter_context(tc.tile_pool(name="rhs", bufs=3))
        sc_pool = ctx.enter_context(tc.tile_pool(name="scratch", bufs=3))
        psA_pool = ctx.enter_context(tc.tile_pool(name="psA", bufs=2, space="PSUM"))
        psB_pool = ctx.enter_context(tc.tile_pool(name="psB", bufs=2, space="PSUM"))
        acc_pool = ctx.enter_context(tc.tile_pool(name="acc", bufs=1))

        ones2 = const.tile([2, P], bf16)
        nc.any.memset(ones2[:], 1.0)
        mbias = const.tile([P, 1], fp32)
        nc.any.memset(mbias[:], MARGIN)
        bias_t = const.tile([P, rt_n], fp32)
        nc.sync.dma_start(out=bias_t[:], in_=bias[:])
        labl_t = const.tile([10, rpc], bf16)
        nc.sync.dma_start(out=labl_t[:], in_=labl[:])
        lhs_t = lhs_pool.tile([P, kch, rpc], bf16)
        nc.sync.dma_start(out=lhs_t[:], in_=lhsT.rearrange("(c p) m -> p c m", p=P))

        acct_t = acc_pool.tile([P, nblk], fp32)
        accw_t = acc_pool.tile([P, nblk], fp32)

        qT_r = qT.rearrange("(c p) n -> p c n", p=P)

        for w in range(nwin_run):
            cs = slice(w * WINW, (w + 1) * WINW)
            rhs_t = rhs_pool.tile([P, kch, WINW], bf16, tag="rhs")
            nc.sync.dma_start(out=rhs_t[:], in_=qT_r[:, :, cs])
            sqf_t = rhs_pool.tile([2, WINW], bf16, tag="sqf")
            nc.sync.dma_start(out=sqf_t[:], in_=sqf[:, cs])
            labr_t = rhs_pool.tile([10, WINW], bf16, tag="labr")
            nc.sync.dma_start(out=labr_t[:], in_=labr[:, cs])
            for r in range(rt_n):
                blk = w * rt_n + r
                ms = slice(r * P, (r + 1) * P)
                psA = psA_pool.tile([P, WINW], fp32, tag="psA")
                psB = psB_pool.tile([P, WINW], fp32, tag="psB")
                for c in range(kch):
                    nc.tensor.matmul(
                        psA[:], lhs_t[:, c, ms], rhs_t[:, c, :],
                        start=(c == 0), stop=False,
                    )
                nc.tensor.matmul(psA[:], ones2[:], sqf_t[:], start=False, stop=True)
                nc.tensor.matmul(psB[:], labl_t[:, ms], labr_t[:], start=True, stop=True)

                d2 = sc_pool.tile([P, WINW], fp32, tag="d2")
                nc.scalar.activation(
                    d2[:], psA[:], Act.Relu,
                    bias=bias_t[:, r : r + 1], scale=-1.0 / dfeat,
                )
                t_ = sc_pool.tile([P, WINW], fp32, tag="t")
                nc.scalar.activation(
                    t_[:], d2[:], Act.Relu,
                    bias=mbias[:], scale=-1.0,
                    accum_out=acct_t[:, blk : blk + 1],
                )
                u_ = sc_pool.tile([P, WINW], fp32, tag="u")
                nc.vector.scalar_tensor_tensor(
                    u_[:], in0=t_[:], scalar=-1.0, in1=d2[:],
                    op0=Alu.mult, op1=Alu.add,
                )
                y_ = sc_pool.tile([P, WINW], fp32, tag="y")
                nc.vector.scalar_tensor_tensor(
                    y_[:], in0=psB[:], scalar=-LAM, in1=u_[:],
                    op0=Alu.mult, op1=Alu.add,
                )
                w_ = sc_pool.tile([P, WINW], fp32, tag="w")
                nc.vector.tensor_scalar(
                    w_[:], y_[:], 0.0, None, Alu.max, Alu.add,
                    accum_out=accw_t[:, blk : blk + 1],
                )

        nc.sync.dma_start(out=acct_d[:], in_=acct_t[:])
        nc.sync.dma_start(out=accw_d[:], in_=accw_t[:])
    nc.compile()
    return nc


def host_prep(predict, gt, b=B, dfeat=DFEAT, ncores=NCORES):
    """Build per-core input maps + the host-side label-pair count."""
    p = np.asarray(predict, np.float32).reshape(b, dfeat)
    q = p * np.float32(math.sqrt(2.0))
    qb = q.astype(BF16)
    qf = qb.astype(np.float32)
    stil = (0.5 * np.einsum("ij,ij->i", qf.astype(np.float64), qf.astype(np.float64))).astype(np.float32)
    smc = (stil - np.float32(C_OFF)).astype(np.float32)
    s_hi = smc.astype(BF16)
    s_lo = (smc - s_hi.astype(np.float32)).astype(BF16)
    sqf_full = np.stack([-s_hi, -s_lo]).astype(BF16)          # [2, B]

    g = np.asarray(gt).reshape(-1).astype(np.int64)
    da, db_, dc = g % 10, (g // 10) % 10, g // 100
    onesb = np.ones(b, np.float32)
    labr_full = np.stack(
        [onesb, da, da * da, onesb, db_, db_ * db_, onesb, dc, dc * dc, onesb]
    ).astype(BF16)                                            # [10, B]
    labl_full = np.stack(
        [da * da, -2.0 * da, onesb, db_ * db_, -2.0 * db_, onesb,
         dc * dc, -2.0 * dc, onesb, np.full(b, -2.0 / LAM, np.float32)]
    ).astype(BF16)                                            # [10, B]
    bias_full = ((stil + np.float32(C_OFF)) / np.float32(dfeat)).astype(np.float32)

    qT = np.ascontiguousarray(qb.T)                           # [D, B] bf16
    rpc = b // ncores
    rt_n = rpc // P
    in_maps = []
    for k in range(ncores):
        rs = slice(k * rpc, (k + 1) * rpc)
        in_maps.append({
            "qT": qT,
            "lhsT": np.ascontiguousarray(qT[:, rs]),
            "sqf": sqf_full,
            "labr": labr_full,
            "labl": np.ascontiguousarray(labl_full[:, rs]),
            "bias": np.ascontiguousarray(bias_full[rs].reshape(rt_n, P).T),
        })
    n_label = int((np.bincount(g) ** 2).sum())
    return in_maps, n_label


def finish(results, n_label, b=B):
    s = 0.0
    for r in results:
        s += float(r["acc_t"].astype(np.float64).sum())
        s += float(r["acc_w"].astype(np.float64).sum())
    loss = (s - 2.0 * n_label) / (float(b) * (b - 1))
    return np.float32(loss)


# ---------------------------------------------------------------------------
# tri2: host-exact label correction + resident lhs + t-only device kernel.
#
# Device computes ONLY  T = sum relu(2 - d2hat)  over a uniform 17-entry
# block cover of the pairwise matrix (diag blocks computed full, halved on
# host).  Label pairs (~45K of 33.5M) are corrected EXACTLY on the host:
#   loss*N = sum_{j>i} [(1-lab)*t + lab*d2]
#          = T_strict + sum_lab (d2_ref - t_hat)
# where t_hat is recomputed on host from the same fp8-quantized q-hat the
# device used, so the subtraction removes exactly what the device added.
#
# Block cover (16 row/col blocks of 512): core k residents A=2k, B=2k+1.
# Entries: (A,A),(B,B),(A,B) from resident tiles; 6 "dual" streamed windows
# w -> (A,w),(B,w); 2 "single" windows sA -> (A,sA), sB -> (B,sB).
# Cyclic duals D_k = {2k+2..2k+7} mod 16 plus the single table below give an
# exact cover of all 120 off-diagonal block pairs (verified at import).
# ---------------------------------------------------------------------------

TRI2_SA = [9, 11, 12, 15, 0, 2, 5, 6]
TRI2_SB = [8, 10, 13, 14, 1, 3, 4, 7]


def tri2_assignment(ncores=NCORES):
    cores = [(2 * k, 2 * k + 1) for k in range(ncores)]
    duals = [[(2 * k + j) % 16 for j in range(2, 8)] for k in range(ncores)]
    singles = list(zip(TRI2_SA, TRI2_SB))
    # verify exact cover of the 120 off-diagonal block pairs
    edges = []
    for k in range(ncores):
        A, B = cores[k]
        edges.append((A, B))
        for w in duals[k]:
            edges.append(tuple(sorted((A, w))))
            edges.append(tuple(sorted((B, w))))
        edges.append(tuple(sorted((A, singles[k][0]))))
        edges.append(tuple(sorted((B, singles[k][1]))))
    edges = [tuple(sorted(e)) for e in edges]
    assert len(edges) == 120 and len(set(edges)) == 120
    return cores, duals, singles


def tri2_entries(k, cores, duals, singles):
    """Entry list for core k: (lhs_sel, col_block, rhs_src) per entry.
    lhs_sel: 0=A, 1=B. rhs_src: 'A'/'B' resident or stream index 0..7."""
    A, B = cores[k]
    ents = [(0, A, "A"), (1, B, "B"), (0, B, "B")]
    for i, w in enumerate(duals[k]):
        ents.append((0, w, i))
        ents.append((1, w, i))
    ents.append((0, singles[k][0], 6))
    ents.append((1, singles[k][1], 7))
    assert len(ents) == 17
    return ents


def build_tri2_nc(b=B, dfeat=DFEAT, ncores=NCORES):
    import concourse.bacc as bacc
    import concourse.mybir as mybir
    from concourse.tile import TileContext

    fp32 = mybir.dt.float32
    bf16 = mybir.dt.bfloat16
    qdt = mybir.dt.float8e4
    Act = mybir.ActivationFunctionType

    kch = dfeat // P               # 16 contraction chunks of 128
    nent = 17
    rt_n = WINW // P               # 4 row tiles per entry
    nblk = nent * rt_n             # 68 accumulator columns
    nwin = 8                       # streamed column windows per core

    nc = bacc.Bacc()
    # host pre-shuffles to [.., P, kch, WINW] so each partition's DMA run is
    # one contiguous kch*WINW (8KB) block — 512B runs measured only 57GB/s.
    lhsq = nc.declare_dram_parameter("lhsq", [2, P, kch, WINW], qdt, isOutput=False)
    rhs_s = nc.declare_dram_parameter("rhs_s", [2, P, 4, kch, WINW], qdt, isOutput=False)
    sqf_s = nc.declare_dram_parameter("sqf_s", [2, nent * WINW], bf16, isOutput=False)
    bias_d = nc.declare_dram_parameter("bias", [P, 2, rt_n], fp32, isOutput=False)
    acct_d = nc.declare_dram_parameter("acc_t", [P, nblk], fp32, isOutput=True)

    # entry structure (identical on every core): lhs_sel + rhs source kind
    ent_lhs = [0, 1, 0] + [0, 1] * 6 + [0, 1]
    ent_rhs = ["A", "B", "B"] + [i // 2 for i in range(12)] + [6, 7]

    with TileContext(nc) as tc, ExitStack() as ctx:
        const = ctx.enter_context(tc.tile_pool(name="const", bufs=1))
        res_pool = ctx.enter_context(tc.tile_pool(name="res", bufs=1))
        win_pool = ctx.enter_context(tc.tile_pool(name="wins", bufs=1))
        sc_pool = ctx.enter_context(tc.tile_pool(name="scratch", bufs=3))
        psA_pool = ctx.enter_context(tc.tile_pool(name="psA", bufs=4, space="PSUM"))
        acc_pool = ctx.enter_context(tc.tile_pool(name="acc", bufs=1))

        ones2 = const.tile([2, P], bf16)
        nc.any.memset(ones2[:], 1.0)

        # warm the PE HAM clock gate while the first lhs DMA is in flight:
        # ~30 constant matmuls keep the array busy past the 3.4us window so
        # the first real matmuls run at 2.4GHz instead of 1.2.
        warm_ps = psA_pool.tile([P, P], fp32, tag="warm")
        for i in range(30):
            nc.tensor.matmul(
                warm_ps[:], ones2[:], ones2[:, 0:P],
                start=(i == 0), stop=(i == 29),
            )

        lhsA_t = res_pool.tile([P, kch, WINW], qdt, tag="lhsA")
        nc.sync.dma_start(out=lhsA_t[:, 0 : kch // 2, :], in_=lhsq[0, :, 0 : kch // 2, :])
        nc.sync.dma_start(out=lhsA_t[:, kch // 2 :, :], in_=lhsq[0, :, kch // 2 :, :])
        sqf_t = const.tile([2, nent * WINW], bf16)
        nc.sync.dma_start(out=sqf_t[:], in_=sqf_s[:])
        bias_t = const.tile([P, 2, rt_n], fp32)
        nc.sync.dma_start(out=bias_t[:], in_=bias_d[:])
        lhsB_t = res_pool.tile([P, kch, WINW], qdt, tag="lhsB")
        nc.sync.dma_start(out=lhsB_t[:], in_=lhsq[1])

        win_g = []
        for i in range(2):
            wt = win_pool.tile([P, 4, kch, WINW], qdt, tag=f"wing{i}")
            nc.sync.dma_start(out=wt[:], in_=rhs_s[i])
            win_g.append(wt)

        acct_t = acc_pool.tile([P, nblk], fp32)

        def rhs_slice(src, c):
            if src == "A":
                return lhsA_t[:, c : c + 2, :]
            if src == "B":
                return lhsB_t[:, c : c + 2, :]
            return win_g[src // 4][:, src % 4, c : c + 2, :]

        for e in range(17):
            lhs_t = lhsA_t if ent_lhs[e] == 0 else lhsB_t
            src = ent_rhs[e]
            sqf_e = sqf_t[:, e * WINW : (e + 1) * WINW]
            for r in range(rt_n):
                blk = e * rt_n + r
                ms = slice(r * P, (r + 1) * P)
                psA = psA_pool.tile([P, WINW], fp32, tag="psA")
                for c in range(0, kch, 2):
                    nc.tensor.matmul(
                        psA[:], lhs_t[:, c : c + 2, ms], rhs_slice(src, c),
                        start=(c == 0), stop=False,
                        perf_mode=mybir.MatmulPerfMode.DoubleRow,
                    )
                nc.tensor.matmul(psA[:], ones2[:], sqf_e, start=False, stop=True)
                t_ = sc_pool.tile([P, WINW], fp32, tag="t")
                nc.scalar.activation(
                    t_[:], psA[:], Act.Relu,
                    bias=bias_t[:, ent_lhs[e], r : r + 1], scale=1.0 / dfeat,
                    accum_out=acct_t[:, blk : blk + 1],
                )

        nc.sync.dma_start(out=acct_d[:], in_=acct_t[:])
    nc.compile()
    return nc


def host_prep_tri2(predict, gt, b=B, dfeat=DFEAT, ncores=NCORES):
    p = np.asarray(predict, np.float32).reshape(b, dfeat)
    q = p * np.float32(math.sqrt(2.0))
    qb = q.astype(ml_dtypes.float8_e4m3)
    # true norms (debias): stil_i = ||q_i||^2 / 2 in f64 -> f32
    stil = (0.5 * np.einsum("ij,ij->i", q.astype(np.float64), q.astype(np.float64))).astype(np.float32)
    smc = (stil - np.float32(C_OFF)).astype(np.float32)
    s_hi = smc.astype(BF16)
    s_lo = (smc - s_hi.astype(np.float32)).astype(BF16)
    sqf_full = np.stack([-s_hi, -s_lo]).astype(BF16)          # [2, B]
    # scalar-activation bias per row: t = relu(psA/D + bias_row)
    bias_full = (np.float32(2.0) - (stil + np.float32(C_OFF)) / np.float32(dfeat)).astype(np.float32)

    qT = np.ascontiguousarray(qb.T)                           # [D, B] fp8
    kch = dfeat // P

    def win_block(v):
        """[P, kch, WINW] with [p,c,n] = qT[c*128+p, v*512+n] — contiguous
        8KB per partition for full-rate DMA."""
        blk = qT[:, v * WINW:(v + 1) * WINW].reshape(kch, P, WINW)
        return np.ascontiguousarray(blk.transpose(1, 0, 2))

    cores, duals, singles = tri2_assignment(ncores)
    rt_n = WINW // P
    in_maps = []
    for k in range(ncores):
        A, Bb = cores[k]
        ents = tri2_entries(k, cores, duals, singles)
        wins = duals[k] + [singles[k][0], singles[k][1]]
        lhs_sx = np.stack([win_block(v) for v in (A, Bb)])
        # [2, P, 4, kch, WINW]: two groups of 4 windows, window axis inside
        # the partition axis so each partition's group DMA run is contiguous
        rhs_sx = np.stack([
            np.stack([win_block(w) for w in wins[g * 4:(g + 1) * 4]], axis=1)
            for g in range(2)])
        sqf_sx = np.concatenate(
            [sqf_full[:, cb * WINW:(cb + 1) * WINW] for _, cb, _ in ents], axis=1)
        bias_sx = np.stack([
            np.ascontiguousarray(bias_full[v * WINW:(v + 1) * WINW].reshape(rt_n, P).T)
            for v in (A, Bb)], axis=1)                        # [P, 2, rt_n]
        in_maps.append({
            "lhsq": np.ascontiguousarray(lhs_sx),
            "rhs_s": np.ascontiguousarray(rhs_sx),
            "sqf_s": np.ascontiguousarray(sqf_sx),
            "bias": np.ascontiguousarray(bias_sx),
        })

    # ---- host-side exact corrections --------------------------------------
    qf64 = qb.astype(np.float64)
    stil64 = stil.astype(np.float64)
    # diagonal cells: t_ii = relu(2 - (2*stil_i - ||qhat_i||^2)/D)
    nq8 = np.einsum("ij,ij->i", qf64, qf64)
    t_ii = np.maximum(2.0 - (2.0 * stil64 - nq8) / dfeat, 0.0)
    tii_blocks = t_ii.reshape(16, WINW).sum(axis=1)           # per diag block v

    # label pairs: subtract device t_hat exactly, add reference d2 exactly
    g = np.asarray(gt).reshape(-1).astype(np.int64)
    p64 = p.astype(np.float64)
    corr = 0.0
    for c in np.unique(g):
        idx = np.nonzero(g == c)[0]
        if len(idx) < 2:
            continue
        Qc = qf64[idx]
        sc = stil64[idx]
        d2hat = (sc[:, None] + sc[None, :] - Qc @ Qc.T) / dfeat
        t_hat = np.maximum(2.0 - d2hat, 0.0)
        Pc = p64[idx]
        sq = np.einsum("ij,ij->i", Pc, Pc)
        d2ref = np.maximum(sq[:, None] + sq[None, :] - 2.0 * (Pc @ Pc.T), 0.0) / dfeat
        iu = np.triu_indices(len(idx), k=1)
        corr += float((d2ref[iu] - t_hat[iu]).sum())
    return in_maps, tii_blocks, corr


def finish_tri2(results, tii_blocks, corr, b=B, ncores=NCORES):
    cores, duals, singles = tri2_assignment(ncores)
    rt_n = WINW // P
    T_strict = 0.0
    for k, r in enumerate(results):
        acc = r["acc_t"].astype(np.float64)                   # [P, 68]
        ent_sums = acc.sum(axis=0).reshape(17, rt_n).sum(axis=1)
        A, Bb = cores[k]
        T_strict += (ent_sums[0] - tii_blocks[A]) / 2.0       # (A,A) diag
        T_strict += (ent_sums[1] - tii_blocks[Bb]) / 2.0      # (B,B) diag
        T_strict += ent_sums[2:].sum()                        # off-diag blocks
    loss = 2.0 * (T_strict + corr) / (float(b) * (b - 1))
    return np.float32(loss)


def kernel(predict, gt):
    global LAST_EXEC_NS, LAST_RESULTS
    from concourse.bass_utils import run_bass_kernel_spmd

    nc = build_tri2_nc()
    in_maps, tii_blocks, corr = host_prep_tri2(predict, gt)
    res = run_bass_kernel_spmd(nc, in_maps, list(range(NCORES)))
    LAST_EXEC_NS = res.exec_time_ns
    LAST_RESULTS = res
    return finish_tri2(res.results, tii_blocks, corr)



# revision 17
# speedup vs baseline: 1.1902x; 1.1902x over previous
"""ContrastiveLoss Trainium2 kernel (8 NeuronCores, SPMD).

Math: reference computes, over p = predict.reshape(B, D):
    d2_ij  = max(||p_i||^2 + ||p_j||^2 - 2 p_i.p_j, 0) / D
    lab_ij = [g_i == g_j]
    term   = lab*d2 + (1-lab)*relu(M - d2),  M = 2
    loss   = 2 * sum_{j>=i} term / (B*(B-1))

Device formulation (full matrix, diagonal self-cancelling):
  q = bf16(sqrt(2)*p); stil_i = (q_i.q_i)/2  (host fp32)
  PSUM_A = q_i.q_j - (stil_j - C)            (bf16 GEMM K=2048 + K=2 feature GEMM)
  d2     = relu(-PSUM_A/D + (stil_i + C)/D)  == ||p^_i - p^_j||^2/D  >= 0 exactly
  t      = relu(M - d2)                      (scalar engine, accum_out -> col sums)
  PSUM_B = dg2 - 2/LAM  where dg2 = sum_digit (d_i - d_j)^2  (exact bf16 GEMM K=10)
  w      = relu((d2 - t) + 2 - LAM*dg2)      (vector engine, accum_out -> col sums)
  sum over full matrix:  S = sum t + sum w  satisfies
  loss   = (S - 2*N_label) / (B*(B-1)),  N_label = sum_c count_c^2 (ordered, incl diag)
Each core owns 1024 rows x all 8192 cols; host sums 8x2 [128,128] accumulators.
"""

import os
import math
from contextlib import ExitStack

import numpy as np
import ml_dtypes

BF16 = ml_dtypes.bfloat16

B = 8192
DFEAT = 2048
NCORES = 8
P = 128
WINW = 512
MARGIN = 2.0
C_OFF = 2048.0
LAM = 64.0

USE_FP8 = True

LAST_EXEC_NS = None
LAST_RESULTS = None


def tri_entries(ncores=NCORES, nblks=16):
    """Per-core list of (row_block, col_block) in 512-units: 2 diagonal entries
    first, then 15 plain upper-triangle entries. Uniform length for every core."""
    per_core = []
    for k in range(ncores):
        a, bb = k, nblks - 1 - k
        ents = [(a, a), (bb, bb)]
        ents += [(a, c) for c in range(a + 1, nblks)]
        ents += [(bb, c) for c in range(bb + 1, nblks)]
        assert len(ents) == nblks + 1
        per_core.append(ents)
    return per_core


def build_tri_nc(b=B, dfeat=DFEAT, ncores=NCORES, use_fp8=False):
    """Triangle variant: 17 host-packed (lhs, rhs) stream entries per core,
    4 row-tiles each; entries 0,1 are diagonal blocks masked to j > i.
    use_fp8: main GEMM in float8e4 with DoubleRow (2 K-chunks per matmul)."""
    import concourse.bacc as bacc
    import concourse.mybir as mybir
    from concourse.tile import TileContext

    fp32 = mybir.dt.float32
    bf16 = mybir.dt.bfloat16
    qdt = mybir.dt.float8e4 if use_fp8 else bf16
    Act = mybir.ActivationFunctionType
    Alu = mybir.AluOpType

    kch = dfeat // P               # contraction chunks
    nent = (b // WINW) + 1         # 17 stream entries
    rt_n = WINW // P               # 4 row tiles per entry
    nblk = nent * rt_n             # 68 accumulator columns

    nc = bacc.Bacc()
    lhs_s = nc.declare_dram_parameter("lhs_s", [nent, dfeat, WINW], qdt, isOutput=False)
    rhs_s = nc.declare_dram_parameter("rhs_s", [nent, dfeat, WINW], qdt, isOutput=False)
    sqf_s = nc.declare_dram_parameter("sqf_s", [nent, 2, WINW], bf16, isOutput=False)
    labr_s = nc.declare_dram_parameter("labr_s", [nent, 10, WINW], bf16, isOutput=False)
    labl_s = nc.declare_dram_parameter("labl_s", [nent, 10, WINW], bf16, isOutput=False)
    bias_s = nc.declare_dram_parameter("bias_s", [nent, P, rt_n], fp32, isOutput=False)
    acct_d = nc.declare_dram_parameter("acc_t", [P, nblk], fp32, isOutput=True)
    accw_d = nc.declare_dram_parameter("acc_w", [P, nblk], fp32, isOutput=True)

    with TileContext(nc) as tc, ExitStack() as ctx:
        const = ctx.enter_context(tc.tile_pool(name="const", bufs=1))
        str_pool = ctx.enter_context(tc.tile_pool(name="streams", bufs=3))
        sc_pool = ctx.enter_context(tc.tile_pool(name="scratch", bufs=3))
        psA_pool = ctx.enter_context(tc.tile_pool(name="psA", bufs=2, space="PSUM"))
        psB_pool = ctx.enter_context(tc.tile_pool(name="psB", bufs=2, space="PSUM"))
        acc_pool = ctx.enter_context(tc.tile_pool(name="acc", bufs=1))

        ones2 = const.tile([2, P], bf16)
        nc.any.memset(ones2[:], 1.0)
        mbias = const.tile([P, 1], fp32)
        nc.any.memset(mbias[:], MARGIN)
        acct_t = acc_pool.tile([P, nblk], fp32)
        accw_t = acc_pool.tile([P, nblk], fp32)

        lhs_r = lhs_s.rearrange("e (c p) n -> e p c n", p=P)
        rhs_r = rhs_s.rearrange("e (c p) n -> e p c n", p=P)

        for e in range(nent):
            lhs_t = str_pool.tile([P, kch, WINW], qdt, tag="lhs")
            nc.sync.dma_start(out=lhs_t[:], in_=lhs_r[e])
            rhs_t = str_pool.tile([P, kch, WINW], qdt, tag="rhs")
            nc.sync.dma_start(out=rhs_t[:], in_=rhs_r[e])
            sqf_t = str_pool.tile([2, WINW], bf16, tag="sqf")
            nc.sync.dma_start(out=sqf_t[:], in_=sqf_s[e])
            labr_t = str_pool.tile([10, WINW], bf16, tag="labr")
            nc.sync.dma_start(out=labr_t[:], in_=labr_s[e])
            labl_t = str_pool.tile([10, WINW], bf16, tag="labl")
            nc.sync.dma_start(out=labl_t[:], in_=labl_s[e])
            bias_t = str_pool.tile([P, rt_n], fp32, tag="bias")
            nc.sync.dma_start(out=bias_t[:], in_=bias_s[e])
            for r in range(rt_n):
                blk = e * rt_n + r
                ms = slice(r * P, (r + 1) * P)
                psA = psA_pool.tile([P, WINW], fp32, tag="psA")
                psB = psB_pool.tile([P, WINW], fp32, tag="psB")
                if use_fp8:
                    for c in range(0, kch, 2):
                        nc.tensor.matmul(
                            psA[:], lhs_t[:, c : c + 2, ms], rhs_t[:, c : c + 2, :],
                            start=(c == 0), stop=False,
                            perf_mode=mybir.MatmulPerfMode.DoubleRow,
                        )
                else:
                    for c in range(kch):
                        nc.tensor.matmul(
                            psA[:], lhs_t[:, c, ms], rhs_t[:, c, :],
                            start=(c == 0), stop=False,
                        )
                nc.tensor.matmul(psA[:], ones2[:], sqf_t[:], start=False, stop=True)
                nc.tensor.matmul(psB[:], labl_t[:, ms], labr_t[:], start=True, stop=True)

                d2 = sc_pool.tile([P, WINW], fp32, tag="d2")
                nc.scalar.activation(
                    d2[:], psA[:], Act.Relu,
                    bias=bias_t[:, r : r + 1], scale=-1.0 / dfeat,
                )
                if e < 2:
                    # diagonal block: keep strictly-upper cells (col > row),
                    # zero the rest; host corrects the t=M per zeroed cell.
                    d2m = sc_pool.tile([P, WINW], fp32, tag="d2m")
                    nc.gpsimd.affine_select(
                        d2m[:], d2[:], pattern=[[1, WINW]],
                        compare_op=Alu.is_gt, fill=0.0,
                        base=-(r * P), channel_multiplier=-1,
                    )
                    d2 = d2m
                t_ = sc_pool.tile([P, WINW], fp32, tag="t")
                nc.scalar.activation(
                    t_[:], d2[:], Act.Relu,
                    bias=mbias[:], scale=-1.0,
                    accum_out=acct_t[:, blk : blk + 1],
                )
                u_ = sc_pool.tile([P, WINW], fp32, tag="u")
                nc.vector.scalar_tensor_tensor(
                    u_[:], in0=t_[:], scalar=-1.0, in1=d2[:],
                    op0=Alu.mult, op1=Alu.add,
                )
                y_ = sc_pool.tile([P, WINW], fp32, tag="y")
                nc.vector.scalar_tensor_tensor(
                    y_[:], in0=psB[:], scalar=-LAM, in1=u_[:],
                    op0=Alu.mult, op1=Alu.add,
                )
                w_ = sc_pool.tile([P, WINW], fp32, tag="w")
                nc.vector.tensor_scalar(
                    w_[:], y_[:], 0.0, None, Alu.max, Alu.add,
                    accum_out=accw_t[:, blk : blk + 1],
                )

        nc.sync.dma_start(out=acct_d[:], in_=acct_t[:])
        nc.sync.dma_start(out=accw_d[:], in_=accw_t[:])
    nc.compile()
    return nc


def host_prep_tri(predict, gt, b=B, dfeat=DFEAT, ncores=NCORES, use_fp8=False):
    """Per-core input maps for the triangle variant + host correction counts.

    use_fp8: quantize q to float8_e4m3 and debias the squared norms by the
    known quantization error energy, so E[d2_hat] == d2 (the raw fp8
    ||p^_i - p^_j||^2 overshoots by (||e_i||^2+||e_j||^2)/D otherwise)."""
    p = np.asarray(predict, np.float32).reshape(b, dfeat)
    q = p * np.float32(math.sqrt(2.0))
    qb = q.astype(ml_dtypes.float8_e4m3 if use_fp8 else BF16)
    qf = qb.astype(np.float32)
    if use_fp8:
        # true norms: cancels the row-common part of the quantization error
        # (E[q.e] != 0 for coarse RN grids); diagonal cells are masked anyway.
        stil = (0.5 * np.einsum("ij,ij->i", q.astype(np.float64), q.astype(np.float64))).astype(np.float32)
    else:
        stil = (0.5 * np.einsum("ij,ij->i", qf.astype(np.float64), qf.astype(np.float64))).astype(np.float32)
    smc = (stil - np.float32(C_OFF)).astype(np.float32)
    s_hi = smc.astype(BF16)
    s_lo = (smc - s_hi.astype(np.float32)).astype(BF16)
    sqf_full = np.stack([-s_hi, -s_lo]).astype(BF16)

    g = np.asarray(gt).reshape(-1).astype(np.int64)
    da, db_, dc = g % 10, (g // 10) % 10, g // 100
    onesb = np.ones(b, np.float32)
    labr_full = np.stack(
        [onesb, da, da * da, onesb, db_, db_ * db_, onesb, dc, dc * dc, onesb]
    ).astype(BF16)
    labl_full = np.stack(
        [da * da, -2.0 * da, onesb, db_ * db_, -2.0 * db_, onesb,
         dc * dc, -2.0 * dc, onesb, np.full(b, -2.0 / LAM, np.float32)]
    ).astype(BF16)
    bias_full = ((stil + np.float32(C_OFF)) / np.float32(dfeat)).astype(np.float32)
    qT = np.ascontiguousarray(qb.T)

    nblks = b // WINW
    del q, qf
    rt_n = WINW // P
    entries = tri_entries(ncores, nblks)
    in_maps = []
    for k in range(ncores):
        ents = entries[k]
        lhs_sx = np.stack([qT[:, rb * WINW:(rb + 1) * WINW] for rb, _ in ents])
        rhs_sx = np.stack([qT[:, cb * WINW:(cb + 1) * WINW] for _, cb in ents])
        sqf_sx = np.stack([sqf_full[:, cb * WINW:(cb + 1) * WINW] for _, cb in ents])
        labr_sx = np.stack([labr_full[:, cb * WINW:(cb + 1) * WINW] for _, cb in ents])
        labl_sx = np.stack([labl_full[:, rb * WINW:(rb + 1) * WINW] for rb, _ in ents])
        bias_sx = np.stack([
            np.ascontiguousarray(bias_full[rb * WINW:(rb + 1) * WINW].reshape(rt_n, P).T)
            for rb, _ in ents])
        in_maps.append({
            "lhs_s": np.ascontiguousarray(lhs_sx),
            "rhs_s": np.ascontiguousarray(rhs_sx),
            "sqf_s": np.ascontiguousarray(sqf_sx),
            "labr_s": np.ascontiguousarray(labr_sx),
            "labl_s": np.ascontiguousarray(labl_sx),
            "bias_s": np.ascontiguousarray(bias_sx),
        })
    n_label = int((np.bincount(g) ** 2).sum())
    n_masked = ncores * 2 * (WINW * (WINW + 1) // 2)   # j<=i cells zeroed per diag entry
    return in_maps, n_label, n_masked


def finish_tri(results, n_label, n_masked, b=B):
    s = 0.0
    for r in results:
        s += float(r["acc_t"].astype(np.float64).sum())
        s += float(r["acc_w"].astype(np.float64).sum())
    n_lab_strict = (n_label - b) // 2
    s_strict = s - MARGIN * n_masked - 2.0 * n_lab_strict
    loss = 2.0 * s_strict / (float(b) * (b - 1))
    return np.float32(loss)


def build_nc(b=B, dfeat=DFEAT, ncores=NCORES, nwin_override=None):
    import concourse.bass as bass
    import concourse.bacc as bacc
    import concourse.mybir as mybir
    from concourse.tile import TileContext

    fp32 = mybir.dt.float32
    bf16 = mybir.dt.bfloat16
    Act = mybir.ActivationFunctionType
    Alu = mybir.AluOpType

    rpc = b // ncores              # rows per core
    rt_n = rpc // P                # row tiles per core
    nwin = b // WINW               # column windows
    nwin_run = nwin if nwin_override is None else nwin_override
    kch = dfeat // P               # contraction chunks
    nblk = nwin * rt_n

    nc = bacc.Bacc()
    qT = nc.declare_dram_parameter("qT", [dfeat, b], bf16, isOutput=False)
    lhsT = nc.declare_dram_parameter("lhsT", [dfeat, rpc], bf16, isOutput=False)
    sqf = nc.declare_dram_parameter("sqf", [2, b], bf16, isOutput=False)
    labr = nc.declare_dram_parameter("labr", [10, b], bf16, isOutput=False)
    labl = nc.declare_dram_parameter("labl", [10, rpc], bf16, isOutput=False)
    bias = nc.declare_dram_parameter("bias", [P, rt_n], fp32, isOutput=False)
    acct_d = nc.declare_dram_parameter("acc_t", [P, nblk], fp32, isOutput=True)
    accw_d = nc.declare_dram_parameter("acc_w", [P, nblk], fp32, isOutput=True)

    with TileContext(nc) as tc, ExitStack() as ctx:
        const = ctx.enter_context(tc.tile_pool(name="const", bufs=1))
        lhs_pool = ctx.enter_context(tc.tile_pool(name="lhs", bufs=1))
        rhs_pool = ctx.enter_context(tc.tile_pool(name="rhs", bufs=3))
        sc_pool = ctx.enter_context(tc.tile_pool(name="scratch", bufs=3))
        psA_pool = ctx.enter_context(tc.tile_pool(name="psA", bufs=2, space="PSUM"))
        psB_pool = ctx.enter_context(tc.tile_pool(name="psB", bufs=2, space="PSUM"))
        acc_pool = ctx.enter_context(tc.tile_pool(name="acc", bufs=1))

        ones2 = const.tile([2, P], bf16)
        nc.any.memset(ones2[:], 1.0)
        mbias = const.tile([P, 1], fp32)
        nc.any.memset(mbias[:], MARGIN)
        bias_t = const.tile([P, rt_n], fp32)
        nc.sync.dma_start(out=bias_t[:], in_=bias[:])
        labl_t = const.tile([10, rpc], bf16)
        nc.sync.dma_start(out=labl_t[:], in_=labl[:])
        lhs_t = lhs_pool.tile([P, kch, rpc], bf16)
        nc.sync.dma_start(out=lhs_t[:], in_=lhsT.rearrange("(c p) m -> p c m", p=P))

        acct_t = acc_pool.tile([P, nblk], fp32)
        accw_t = acc_pool.tile([P, nblk], fp32)

        qT_r = qT.rearrange("(c p) n -> p c n", p=P)

        for w in range(nwin_run):
            cs = slice(w * WINW, (w + 1) * WINW)
            rhs_t = rhs_pool.tile([P, kch, WINW], bf16, tag="rhs")
            nc.sync.dma_start(out=rhs_t[:], in_=qT_r[:, :, cs])
            sqf_t = rhs_pool.tile([2, WINW], bf16, tag="sqf")
            nc.sync.dma_start(out=sqf_t[:], in_=sqf[:, cs])
            labr_t = rhs_pool.tile([10, WINW], bf16, tag="labr")
            nc.sync.dma_start(out=labr_t[:], in_=labr[:, cs])
            for r in range(rt_n):
                blk = w * rt_n + r
                ms = slice(r * P, (r + 1) * P)
                psA = psA_pool.tile([P, WINW], fp32, tag="psA")
                psB = psB_pool.tile([P, WINW], fp32, tag="psB")
                for c in range(kch):
                    nc.tensor.matmul(
                        psA[:], lhs_t[:, c, ms], rhs_t[:, c, :],
                        start=(c == 0), stop=False,
                    )
                nc.tensor.matmul(psA[:], ones2[:], sqf_t[:], start=False, stop=True)
                nc.tensor.matmul(psB[:], labl_t[:, ms], labr_t[:], start=True, stop=True)

                d2 = sc_pool.tile([P, WINW], fp32, tag="d2")
                nc.scalar.activation(
                    d2[:], psA[:], Act.Relu,
                    bias=bias_t[:, r : r + 1], scale=-1.0 / dfeat,
                )
                t_ = sc_pool.tile([P, WINW], fp32, tag="t")
                nc.scalar.activation(
                    t_[:], d2[:], Act.Relu,
                    bias=mbias[:], scale=-1.0,
                    accum_out=acct_t[:, blk : blk + 1],
                )
                u_ = sc_pool.tile([P, WINW], fp32, tag="u")
                nc.vector.scalar_tensor_tensor(
                    u_[:], in0=t_[:], scalar=-1.0, in1=d2[:],
                    op0=Alu.mult, op1=Alu.add,
                )
                y_ = sc_pool.tile([P, WINW], fp32, tag="y")
                nc.vector.scalar_tensor_tensor(
                    y_[:], in0=psB[:], scalar=-LAM, in1=u_[:],
                    op0=Alu.mult, op1=Alu.add,
                )
                w_ = sc_pool.tile([P, WINW], fp32, tag="w")
                nc.vector.tensor_scalar(
                    w_[:], y_[:], 0.0, None, Alu.max, Alu.add,
                    accum_out=accw_t[:, blk : blk + 1],
                )

        nc.sync.dma_start(out=acct_d[:], in_=acct_t[:])
        nc.sync.dma_start(out=accw_d[:], in_=accw_t[:])
    nc.compile()
    return nc


def host_prep(predict, gt, b=B, dfeat=DFEAT, ncores=NCORES):
    """Build per-core input maps + the host-side label-pair count."""
    p = np.asarray(predict, np.float32).reshape(b, dfeat)
    q = p * np.float32(math.sqrt(2.0))
    qb = q.astype(BF16)
    qf = qb.astype(np.float32)
    stil = (0.5 * np.einsum("ij,ij->i", qf.astype(np.float64), qf.astype(np.float64))).astype(np.float32)
    smc = (stil - np.float32(C_OFF)).astype(np.float32)
    s_hi = smc.astype(BF16)
    s_lo = (smc - s_hi.astype(np.float32)).astype(BF16)
    sqf_full = np.stack([-s_hi, -s_lo]).astype(BF16)          # [2, B]

    g = np.asarray(gt).reshape(-1).astype(np.int64)
    da, db_, dc = g % 10, (g // 10) % 10, g // 100
    onesb = np.ones(b, np.float32)
    labr_full = np.stack(
        [onesb, da, da * da, onesb, db_, db_ * db_, onesb, dc, dc * dc, onesb]
    ).astype(BF16)                                            # [10, B]
    labl_full = np.stack(
        [da * da, -2.0 * da, onesb, db_ * db_, -2.0 * db_, onesb,
         dc * dc, -2.0 * dc, onesb, np.full(b, -2.0 / LAM, np.float32)]
    ).astype(BF16)                                            # [10, B]
    bias_full = ((stil + np.float32(C_OFF)) / np.float32(dfeat)).astype(np.float32)

    qT = np.ascontiguousarray(qb.T)                           # [D, B] bf16
    rpc = b // ncores
    rt_n = rpc // P
    in_maps = []
    for k in range(ncores):
        rs = slice(k * rpc, (k + 1) * rpc)
        in_maps.append({
            "qT": qT,
            "lhsT": np.ascontiguousarray(qT[:, rs]),
            "sqf": sqf_full,
            "labr": labr_full,
            "labl": np.ascontiguousarray(labl_full[:, rs]),
            "bias": np.ascontiguousarray(bias_full[rs].reshape(rt_n, P).T),
        })
    n_label = int((np.bincount(g) ** 2).sum())
    return in_maps, n_label


def finish(results, n_label, b=B):
    s = 0.0
    for r in results:
        s += float(r["acc_t"].astype(np.float64).sum())
        s += float(r["acc_w"].astype(np.float64).sum())
    loss = (s - 2.0 * n_label) / (float(b) * (b - 1))
    return np.float32(loss)


# ---------------------------------------------------------------------------
# tri2: host-exact label correction + resident lhs + t-only device kernel.
#
# Device computes ONLY  T = sum relu(2 - d2hat)  over a uniform 17-entry
# block cover of the pairwise matrix (diag blocks computed full, halved on
# host).  Label pairs (~45K of 33.5M) are corrected EXACTLY on the host:
#   loss*N = sum_{j>i} [(1-lab)*t + lab*d2]
#          = T_strict + sum_lab (d2_ref - t_hat)
# where t_hat is recomputed on host from the same fp8-quantized q-hat the
# device used, so the subtraction removes exactly what the device added.
#
# Block cover (16 row/col blocks of 512): core k residents A=2k, B=2k+1.
# Entries: (A,A),(B,B),(A,B) from resident tiles; 6 "dual" streamed windows
# w -> (A,w),(B,w); 2 "single" windows sA -> (A,sA), sB -> (B,sB).
# Cyclic duals D_k = {2k+2..2k+7} mod 16 plus the single table below give an
# exact cover of all 120 off-diagonal block pairs (verified at import).
# ---------------------------------------------------------------------------

TRI2_SA = [9, 11, 12, 15, 0, 2, 5, 6]
TRI2_SB = [8, 10, 13, 14, 1, 3, 4, 7]


def tri2_assignment(ncores=NCORES):
    cores = [(2 * k, 2 * k + 1) for k in range(ncores)]
    duals = [[(2 * k + j) % 16 for j in range(2, 8)] for k in range(ncores)]
    singles = list(zip(TRI2_SA, TRI2_SB))
    # verify exact cover of the 120 off-diagonal block pairs
    edges = []
    for k in range(ncores):
        A, B = cores[k]
        edges.append((A, B))
        for w in duals[k]:
            edges.append(tuple(sorted((A, w))))
            edges.append(tuple(sorted((B, w))))
        edges.append(tuple(sorted((A, singles[k][0]))))
        edges.append(tuple(sorted((B, singles[k][1]))))
    edges = [tuple(sorted(e)) for e in edges]
    assert len(edges) == 120 and len(set(edges)) == 120
    return cores, duals, singles


def tri2_entries(k, cores, duals, singles):
    """Entry list for core k: (lhs_sel, col_block, rhs_src) per entry.
    lhs_sel: 0=A, 1=B. rhs_src: 'A'/'B' resident or stream index 0..7."""
    A, B = cores[k]
    ents = [(0, A, "A"), (1, B, "B"), (0, B, "B")]
    for i, w in enumerate(duals[k]):
        ents.append((0, w, i))
        ents.append((1, w, i))
    ents.append((0, singles[k][0], 6))
    ents.append((1, singles[k][1], 7))
    assert len(ents) == 17
    return ents


def build_tri2_nc(b=B, dfeat=DFEAT, ncores=NCORES):
    import concourse.bacc as bacc
    import concourse.mybir as mybir
    from concourse.tile import TileContext

    fp32 = mybir.dt.float32
    bf16 = mybir.dt.bfloat16
    qdt = mybir.dt.float8e4
    Act = mybir.ActivationFunctionType

    kch = dfeat // P               # 16 contraction chunks of 128
    nent = 17
    rt_n = WINW // P               # 4 row tiles per entry
    nblk = nent * rt_n             # 68 accumulator columns
    nwin = 8                       # streamed column windows per core

    nc = bacc.Bacc()
    # host pre-shuffles to [.., P, kch, WINW] so each partition's DMA run is
    # one contiguous kch*WINW (8KB) block — 512B runs measured only 57GB/s.
    lhsq = nc.declare_dram_parameter("lhsq", [2, P, kch, WINW], qdt, isOutput=False)
    rhs_s = nc.declare_dram_parameter("rhs_s", [2, P, 4, kch, WINW], qdt, isOutput=False)
    sqf_s = nc.declare_dram_parameter("sqf_s", [2, nent * WINW], bf16, isOutput=False)
    bias_d = nc.declare_dram_parameter("bias", [P, 2, rt_n], fp32, isOutput=False)
    acct_d = nc.declare_dram_parameter("acc_t", [P, nblk], fp32, isOutput=True)

    # entry structure (identical on every core): lhs_sel + rhs source kind
    ent_lhs = [0, 1, 0] + [0, 1] * 6 + [0, 1]
    ent_rhs = ["A", "B", "B"] + [i // 2 for i in range(12)] + [6, 7]

    with TileContext(nc) as tc, ExitStack() as ctx:
        const = ctx.enter_context(tc.tile_pool(name="const", bufs=1))
        res_pool = ctx.enter_context(tc.tile_pool(name="res", bufs=1))
        win_pool = ctx.enter_context(tc.tile_pool(name="wins", bufs=1))
        sc_pool = ctx.enter_context(tc.tile_pool(name="scratch", bufs=3))
        psA_pool = ctx.enter_context(tc.tile_pool(name="psA", bufs=4, space="PSUM"))
        acc_pool = ctx.enter_context(tc.tile_pool(name="acc", bufs=1))

        ones2 = const.tile([2, P], bf16)
        nc.any.memset(ones2[:], 1.0)

        # warm the PE HAM clock gate while the first lhs DMA is in flight:
        # ~30 constant matmuls keep the array busy past the 3.4us window so
        # the first real matmuls run at 2.4GHz instead of 1.2.
        warm_ps = psA_pool.tile([P, P], fp32, tag="warm")
        for i in range(30):
            nc.tensor.matmul(
                warm_ps[:], ones2[:], ones2[:, 0:P],
                start=(i == 0), stop=(i == 29),
            )

        lhsA_t = res_pool.tile([P, kch, WINW], qdt, tag="lhsA")
        nc.sync.dma_start(out=lhsA_t[:, 0 : kch // 2, :], in_=lhsq[0, :, 0 : kch // 2, :])
        nc.sync.dma_start(out=lhsA_t[:, kch // 2 :, :], in_=lhsq[0, :, kch // 2 :, :])
        sqf_t = const.tile([2, nent * WINW], bf16)
        nc.sync.dma_start(out=sqf_t[:], in_=sqf_s[:])
        bias_t = const.tile([P, 2, rt_n], fp32)
        nc.sync.dma_start(out=bias_t[:], in_=bias_d[:])
        lhsB_t = res_pool.tile([P, kch, WINW], qdt, tag="lhsB")
        nc.sync.dma_start(out=lhsB_t[:], in_=lhsq[1])

        # flattened [P, 4*kch, WINW] so window slices stay 3D APs — a 4D
        # slice with a kept size-1 dim cost +43ns on every DR matmul
        win_g = []
        for i in range(2):
            wt = win_pool.tile([P, 4 * kch, WINW], qdt, tag=f"wing{i}")
            nc.sync.dma_start(out=wt[:], in_=rhs_s[i])
            win_g.append(wt)

        acct_t = acc_pool.tile([P, nblk], fp32)

        def rhs_slice(src, c):
            if src == "A":
                return lhsA_t[:, c : c + 2, :]
            if src == "B":
                return lhsB_t[:, c : c + 2, :]
            base = (src % 4) * kch + c
            return win_g[src // 4][:, base : base + 2, :]

        for e in range(17):
            lhs_t = lhsA_t if ent_lhs[e] == 0 else lhsB_t
            src = ent_rhs[e]
            sqf_e = sqf_t[:, e * WINW : (e + 1) * WINW]
            for r in range(rt_n):
                blk = e * rt_n + r
                ms = slice(r * P, (r + 1) * P)
                psA = psA_pool.tile([P, WINW], fp32, tag="psA")
                for c in range(0, kch, 2):
                    nc.tensor.matmul(
                        psA[:], lhs_t[:, c : c + 2, ms], rhs_slice(src, c),
                        start=(c == 0), stop=False,
                        perf_mode=mybir.MatmulPerfMode.DoubleRow,
                    )
                nc.tensor.matmul(psA[:], ones2[:], sqf_e, start=False, stop=True)
                t_ = sc_pool.tile([P, WINW], fp32, tag="t")
                nc.scalar.activation(
                    t_[:], psA[:], Act.Relu,
                    bias=bias_t[:, ent_lhs[e], r : r + 1], scale=1.0 / dfeat,
                    accum_out=acct_t[:, blk : blk + 1],
                )

        nc.sync.dma_start(out=acct_d[:], in_=acct_t[:])
    nc.compile()
    return nc


def host_prep_tri2(predict, gt, b=B, dfeat=DFEAT, ncores=NCORES):
    p = np.asarray(predict, np.float32).reshape(b, dfeat)
    q = p * np.float32(math.sqrt(2.0))
    qb = q.astype(ml_dtypes.float8_e4m3)
    # true norms (debias): stil_i = ||q_i||^2 / 2 in f64 -> f32
    stil = (0.5 * np.einsum("ij,ij->i", q.astype(np.float64), q.astype(np.float64))).astype(np.float32)
    smc = (stil - np.float32(C_OFF)).astype(np.float32)
    s_hi = smc.astype(BF16)
    s_lo = (smc - s_hi.astype(np.float32)).astype(BF16)
    sqf_full = np.stack([-s_hi, -s_lo]).astype(BF16)          # [2, B]
    # scalar-activation bias per row: t = relu(psA/D + bias_row)
    bias_full = (np.float32(2.0) - (stil + np.float32(C_OFF)) / np.float32(dfeat)).astype(np.float32)

    qT = np.ascontiguousarray(qb.T)                           # [D, B] fp8
    kch = dfeat // P

    def win_block(v):
        """[P, kch, WINW] with [p,c,n] = qT[c*128+p, v*512+n] — contiguous
        8KB per partition for full-rate DMA."""
        blk = qT[:, v * WINW:(v + 1) * WINW].reshape(kch, P, WINW)
        return np.ascontiguousarray(blk.transpose(1, 0, 2))

    cores, duals, singles = tri2_assignment(ncores)
    rt_n = WINW // P
    in_maps = []
    for k in range(ncores):
        A, Bb = cores[k]
        ents = tri2_entries(k, cores, duals, singles)
        wins = duals[k] + [singles[k][0], singles[k][1]]
        lhs_sx = np.stack([win_block(v) for v in (A, Bb)])
        # [2, P, 4, kch, WINW]: two groups of 4 windows, window axis inside
        # the partition axis so each partition's group DMA run is contiguous
        rhs_sx = np.stack([
            np.stack([win_block(w) for w in wins[g * 4:(g + 1) * 4]], axis=1)
            for g in range(2)])
        sqf_sx = np.concatenate(
            [sqf_full[:, cb * WINW:(cb + 1) * WINW] for _, cb, _ in ents], axis=1)
        bias_sx = np.stack([
            np.ascontiguousarray(bias_full[v * WINW:(v + 1) * WINW].reshape(rt_n, P).T)
            for v in (A, Bb)], axis=1)                        # [P, 2, rt_n]
        in_maps.append({
            "lhsq": np.ascontiguousarray(lhs_sx),
            "rhs_s": np.ascontiguousarray(rhs_sx),
            "sqf_s": np.ascontiguousarray(sqf_sx),
            "bias": np.ascontiguousarray(bias_sx),
        })

    # ---- host-side exact corrections --------------------------------------
    qf64 = qb.astype(np.float64)
    stil64 = stil.astype(np.float64)
    # diagonal cells: t_ii = relu(2 - (2*stil_i - ||qhat_i||^2)/D)
    nq8 = np.einsum("ij,ij->i", qf64, qf64)
    t_ii = np.maximum(2.0 - (2.0 * stil64 - nq8) / dfeat, 0.0)
    tii_blocks = t_ii.reshape(16, WINW).sum(axis=1)           # per diag block v

    # label pairs: subtract device t_hat exactly, add reference d2 exactly
    g = np.asarray(gt).reshape(-1).astype(np.int64)
    p64 = p.astype(np.float64)
    corr = 0.0
    for c in np.unique(g):
        idx = np.nonzero(g == c)[0]
        if len(idx) < 2:
            continue
        Qc = qf64[idx]
        sc = stil64[idx]
        d2hat = (sc[:, None] + sc[None, :] - Qc @ Qc.T) / dfeat
        t_hat = np.maximum(2.0 - d2hat, 0.0)
        Pc = p64[idx]
        sq = np.einsum("ij,ij->i", Pc, Pc)
        d2ref = np.maximum(sq[:, None] + sq[None, :] - 2.0 * (Pc @ Pc.T), 0.0) / dfeat
        iu = np.triu_indices(len(idx), k=1)
        corr += float((d2ref[iu] - t_hat[iu]).sum())
    return in_maps, tii_blocks, corr


def finish_tri2(results, tii_blocks, corr, b=B, ncores=NCORES):
    cores, duals, singles = tri2_assignment(ncores)
    rt_n = WINW // P
    T_strict = 0.0
    for k, r in enumerate(results):
        acc = r["acc_t"].astype(np.float64)                   # [P, 68]
        ent_sums = acc.sum(axis=0).reshape(17, rt_n).sum(axis=1)
        A, Bb = cores[k]
        T_strict += (ent_sums[0] - tii_blocks[A]) / 2.0       # (A,A) diag
        T_strict += (ent_sums[1] - tii_blocks[Bb]) / 2.0      # (B,B) diag
        T_strict += ent_sums[2:].sum()                        # off-diag blocks
    loss = 2.0 * (T_strict + corr) / (float(b) * (b - 1))
    return np.float32(loss)


def kernel(predict, gt):
    global LAST_EXEC_NS, LAST_RESULTS
    from concourse.bass_utils import run_bass_kernel_spmd

    nc = build_tri2_nc()
    in_maps, tii_blocks, corr = host_prep_tri2(predict, gt)
    res = run_bass_kernel_spmd(nc, in_maps, list(range(NCORES)))
    LAST_EXEC_NS = res.exec_time_ns
    LAST_RESULTS = res
    return finish_tri2(res.results, tii_blocks, corr)

